# revision 64
# baseline (speedup 1.0000x reference)
"""Trainium2 Bass kernel for nn_CrossAttention (B=4, Lq=Lk=E=1024, H=16).

Sharding: data-parallel over 8 cores; core c handles batch c//2, query rows
(c%2)*512 ... +512. Heads stay local to a core, so softmax + head-mean need
no collectives. Each core computes a [512,1024] slice of attn_output and
attn_scores.

v2 changes over the first working kernel (129462ns -> 124613ns sim):
  - q-side inverse-RMS folded into the exp scale AP (valid when bq == 0,
    checked at runtime in kernel(); bq is zero for this module), so q tiles
    skip the DVE xn normalization entirely. (The k side must keep xn: the
    HW transpose ignores its "identity" matrix values, so a diag(inv)
    cannot be folded into the transpose.)
  - act tables: a dummy Sqrt at t=0 loads the sqrt+square set under the
    initial DMA wait; a dummy Exp anchored on the last sqrt's output pulls
    the exp-set load into the idle window before the first logits -> 2
    loads instead of 3, neither stalling the square/exp streams.
  - weights pre-packed o-major on the host; x tiles DMA'd first, o0/o1
    weight blocks eagerly, remaining weights stream during attention.
  - remaining o2..o7 projections dripped 2 chunks per attention head
    instead of per-o lumps, so PE never starves the exp stream for ~5us.
  - the no-2x-mode scalar_tensor_tensor acc chains (~1127ns each) for
    it1/it2 are split into DVE tensor_scalar (2x, ~690ns) + GPSIMD
    tensor_tensor add, running on the otherwise-idle Pool engine; it3's
    chain stays on DVE and all its odd heads run in the it1 section so
    acc3 is complete long before the finale.
  - finale merges+transposes pipelined in 256-col quarters; attn_output
    matmuls quartered so the last output DMA waits only on the last one.
  - deeper SBUF rings (et/acc 8, wet 4, stats 24, dg 6, scs/scT 3, osb 5)
    decouple the ACT exp stream from its DVE/Pool consumers (-2us); tpA=2
    and the phase-A scratch rings are saturated (measured).
Measured dead ends kept out: fp8/DoubleRow (accuracy), diag-fold into the
PE transpose (HW ignores the matrix), DMA xbar transposes for scT (+15us
HWDGE serialization), remote-DMA k-projection sharing across core pairs
(TimelineSim cannot model remote sems - deadlock), collective_compute
(15us constant overhead in the cost model).
"""

import numpy as np
from contextlib import ExitStack

B, LQ, LK, E = 4, 1024, 1024, 1024
H = 16
HD = E // H  # 64
N_CORES = 8
QROWS = LQ // 2  # 512 rows of q per core
EPS = 1.1920929e-07

_CACHE = {}


def _build_program(fold_q):
    import concourse.bass as bass
    import concourse.tile as tile
    from concourse import bacc, mybir

    f32 = mybir.dt.float32
    bf16 = mybir.dt.bfloat16
    f16 = mybir.dt.float16
    Alu = mybir.AluOpType
    Act = mybir.ActivationFunctionType

    nc = bacc.Bacc("TRN2", target_bir_lowering=False, debug=False,
                   num_devices=N_CORES)

    xq = nc.dram_tensor("xq", [QROWS, E], bf16, kind="ExternalInput").ap()
    xk = nc.dram_tensor("xk", [LK, E], bf16, kind="ExternalInput").ap()
    vv = nc.dram_tensor("vv", [LK, E], bf16, kind="ExternalInput").ap()
    # weights pre-packed on host to [p, o-tile, e-tile, c] so per-o blocks
    # are contiguous and the o0/o1 blocks can be DMA'd eagerly
    wqt = nc.dram_tensor("wqt", [128, 8, 8, 128], bf16, kind="ExternalInput").ap()
    wkt = nc.dram_tensor("wkt", [128, 8, 8, 128], bf16, kind="ExternalInput").ap()
    bq = nc.dram_tensor("bq", [128, 8], f32, kind="ExternalInput").ap()
    bk = nc.dram_tensor("bk", [128, 8], f32, kind="ExternalInput").ap()
    ident = nc.dram_tensor("ident", [128, 128], bf16, kind="ExternalInput").ap()

    out = nc.dram_tensor("out", [QROWS, E], bf16, kind="ExternalOutput").ap()
    sc = nc.dram_tensor("sc", [QROWS, LK], bf16, kind="ExternalOutput").ap()

    with tile.TileContext(nc) as tc, ExitStack() as ctx:
        const_pool = ctx.enter_context(tc.tile_pool(name="const", bufs=1))
        eps_sb = const_pool.tile([128, 1], f32, name="eps_sb")
        nc.vector.memset(eps_sb[:], EPS)
        epsq_sb = const_pool.tile([128, 1], f32, name="epsq_sb")
        nc.vector.memset(epsq_sb[:], HD * EPS)
        # dummy sqrt so the act-table policy loads the sqrt+square set first
        # (otherwise Square picks the exp set, Sqrt forces a mid-phase-A
        # reload, and Exp forces a third)
        dummy = const_pool.tile([128, 1], f32, name="dummy")
        nc.scalar.activation(dummy[:], eps_sb[:],
                             mybir.ActivationFunctionType.Sqrt)
        id_sb = const_pool.tile([128, 128], bf16)
        id16 = const_pool.tile([128, 128], f16, name="id16")
        bq_sb = const_pool.tile([128, 8], f32, name="bq_sb")
        bk_sb = const_pool.tile([128, 8], f32, name="bk_sb")

        # big persistent SBUF tensors
        big_pool = ctx.enter_context(tc.tile_pool(name="big", bufs=1))
        xk_sb = big_pool.tile([128, 8, E], bf16, name="xk_sb")  # [row%128, row//128, e]
        xq_sb = big_pool.tile([128, 4, E], bf16, name="xq_sb")
        xkT = big_pool.tile([128, 8, LK], bf16, name="xkT")     # [e-part, e-tile, k-row]
        xqT = big_pool.tile([128, 8, QROWS], bf16, name="xqT")  # [e-part, e-tile, q-row]
        kT = big_pool.tile([128, 8, LK], bf16, name="kT")       # [feat-part, o, k-row]
        qT = big_pool.tile([128, 8, QROWS], bf16, name="qT")
        v_sb = big_pool.tile([128, 8, E], bf16, name="v_sb")    # [j-part, j-tile, d]
        wk_sb = big_pool.tile([128, 8, 8, 128], bf16, name="wk_sb")  # [e-p, o, e-t, c]
        wq_sb = big_pool.tile([128, 8, 8, 128], bf16, name="wq_sb")

        stats = ctx.enter_context(tc.tile_pool(name="stats", bufs=24))
        # q-side exp scales must persist from phase A until their it-block
        einv_pool = ctx.enter_context(tc.tile_pool(name="einv", bufs=1))
        einv = [einv_pool.tile([128, 1], f32, name=f"einv{i}") for i in range(4)]

        # PSUM (8 banks), LIFO pool nesting:
        #   sp(2) [ tpA(2)+ppA(2) {phase A} | lg(4) [ ppB(2) {it0+proj}
        #   | tpF(2) {finales} ] ]
        sp_pool = ctx.enter_context(tc.tile_pool(name="sp", bufs=1, space="PSUM"))
        pools = {}

        # ---------------- loads: few big DMAs, priority order ---------------
        def load_tiled(dst, src_dram, t0, t1):
            src = src_dram[t0 * 128:t1 * 128, :].rearrange(
                "(t p) e -> p t e", p=128)
            nc.sync.dma_start(dst[:, t0:t1, :], src)

        load_tiled(xk_sb, xk, 0, 1)
        load_tiled(xk_sb, xk, 1, 2)
        nc.sync.dma_start(id_sb[:], ident[:])
        nc.vector.tensor_copy(id16[:], id_sb[:])
        load_tiled(xk_sb, xk, 2, 4)
        load_tiled(xq_sb, xq, 0, 2)
        load_tiled(xk_sb, xk, 4, 6)
        load_tiled(xk_sb, xk, 6, 8)
        load_tiled(xq_sb, xq, 2, 4)
        nc.sync.dma_start(bq_sb[:], bq[:])
        nc.sync.dma_start(bk_sb[:], bk[:])
        # eager o0/o1 weight blocks so the first attention heads' projections
        # can complete right behind the norm pipeline; the rest stream in
        # during the attention phase
        nc.sync.dma_start(wk_sb[:, 0:2], wkt[:, 0:2])
        nc.sync.dma_start(wq_sb[:, 0:2], wqt[:, 0:2])
        nc.sync.dma_start(wk_sb[:, 2:8], wkt[:, 2:8])
        nc.sync.dma_start(wq_sb[:, 2:8], wqt[:, 2:8])

        pp_k = {}
        pp_q = {}

        def proj_chunk(o, which, t):
            # accumulate projection output columns t*128..t*128+128 for o-tile
            if which == "k":
                w_sb, xT, pp_map = wk_sb, xkT, pp_k
            else:
                w_sb, xT, pp_map = wq_sb, xqT, pp_q
            half, tl = divmod(t, 4)
            if (o, half) not in pp_map:
                pp_map[(o, half)] = pools["pp"].tile(
                    [128, 512], f32, tag="pp", name=f"pp_{which}{o}_{half}")
            pk = pp_map[(o, half)]
            cols = slice(tl * 128, tl * 128 + 128)
            for e in range(8):
                nc.tensor.matmul(
                    pk[:, cols], lhsT=w_sb[:, o, e, :],
                    rhs=xT[:, e, t * 128:(t + 1) * 128],
                    start=(e == 0), stop=(e == 7),
                )

        def proj_evac(o, which, half):
            if which == "k":
                pp_map, dst, b_sb = pp_k, kT, bk_sb
            else:
                pp_map, dst, b_sb = pp_q, qT, bq_sb
            pk = pp_map.pop((o, half))
            cols = slice(half * 512, half * 512 + 512)
            nc.vector.tensor_scalar(
                out=dst[:, o, cols], in0=pk[:],
                scalar1=b_sb[:, o:o + 1], scalar2=None, op0=Alu.add,
            )

        def proj_o(o, which):
            ntiles = 8 if which == "k" else 4
            for t in range(ntiles):
                proj_chunk(o, which, t)
                if t % 4 == 3:
                    proj_evac(o, which, t // 4)

        # attention SBUF pools created BEFORE phase A so they don't recycle
        # phase-A scratch addresses (recycling adds WAW deps on the slow
        # Pool xn chain, which delayed the first exp by ~17us)
        et_pool = ctx.enter_context(tc.tile_pool(name="et", bufs=8))
        dg_pool = ctx.enter_context(tc.tile_pool(name="dg", bufs=6))
        acc_pool = ctx.enter_context(tc.tile_pool(name="acc", bufs=8))
        wet_pool = ctx.enter_context(tc.tile_pool(name="wet", bufs=4))
        scs_pool = ctx.enter_context(tc.tile_pool(name="scs", bufs=3))
        scT_pool = ctx.enter_context(tc.tile_pool(name="scT", bufs=3))
        osb_pool = ctx.enter_context(tc.tile_pool(name="osb", bufs=5))

        # ---------------- phase A: rmsnorm + transpose -----------------------
        with ExitStack() as actx:
            tpA = actx.enter_context(tc.tile_pool(name="tpA", bufs=2, space="PSUM"))
            pools["pp"] = actx.enter_context(
                tc.tile_pool(name="ppA", bufs=2, space="PSUM"))
            sqscr = actx.enter_context(tc.tile_pool(name="sqscr", bufs=3))
            dgn_pool = actx.enter_context(tc.tile_pool(name="dgn", bufs=4))

            def norm_stats(x_sb, t, n, which, it=None):
                # ACT Square+accum -> ACT sqrt -> DVE recip (+ DVE xn for k;
                # the HW transpose ignores its "identity" matrix values so
                # the k-side scale must be applied pre-transpose; folded q
                # tiles skip xn entirely, inv lands in the exp scale AP)
                ssq = stats.tile([128, 1], f32, tag="ssq", name=f"ssq{n}")
                sq = sqscr.tile([128, E], bf16, tag="sq")
                nc.scalar.activation(sq[:], x_sb[:, t, :], Act.Square,
                                     accum_out=ssq[:])
                s = stats.tile([128, 1], f32, tag="s", name=f"s{n}")
                folded = fold_q and which == "q"
                if folded:
                    # s = 8*sqrt(ms) so DVE recip directly yields invq/8
                    nc.scalar.activation(s[:], ssq[:], Act.Sqrt,
                                         bias=epsq_sb[:], scale=float(HD) / E)
                    inv = einv[it]
                else:
                    nc.scalar.activation(s[:], ssq[:], Act.Sqrt,
                                         bias=eps_sb[:], scale=1.0 / E)
                    inv = stats.tile([128, 1], f32, tag="inv", name=f"inv{n}")
                nc.vector.reciprocal(inv[:], s[:])
                if folded:
                    src = x_sb[:, t, :]
                else:
                    xn = dgn_pool.tile([128, E], bf16, tag="xn")
                    nc.vector.tensor_scalar(
                        out=xn[:], in0=x_sb[:, t, :], scalar1=inv[:],
                        scalar2=None, op0=Alu.mult,
                    )
                    src = xn[:]
                return src, s

            def transpose_part(which, t, xT_tile, src):
                tp = tpA.tile([128, 8, 128], bf16, tag="tp")
                for e in range(8):
                    nc.tensor.transpose(
                        tp[:, e, :], src[:, e * 128:(e + 1) * 128], id_sb[:])
                nc.vector.tensor_copy(
                    xT_tile[:, :, t * 128:(t + 1) * 128], tp[:])
                # keep o0/o1 projections flowing behind the transposes
                for o in (0, 1):
                    proj_chunk(o, which, t)
                    if t % 4 == 3:
                        proj_evac(o, which, t // 4)

            # k tiles first so the k-side proj PSUM groups drain through the
            # 2-buf ring before the q groups claim it. One-tile software
            # pipeline skew: engines execute in order, so emitting xn(t+1)
            # before copy(t) keeps DVE from head-of-line blocking on the PE
            # transposes (serial-loop pace ~2.2us/tile -> engine pace ~1.6)
            seq = [("k", 0), ("k", 1), ("k", 2), ("k", 3), ("k", 4), ("k", 5),
                   ("k", 6), ("k", 7), ("q", 0), ("q", 1), ("q", 2), ("q", 3)]
            s_last = None
            pend = None
            for n, (which, t) in enumerate(seq):
                x_sb_, xT_ = (xk_sb, xkT) if which == "k" else (xq_sb, xqT)
                src, s_last = norm_stats(
                    x_sb_, t, n, which, it=t if which == "q" else None)
                if pend is not None:
                    transpose_part(*pend)
                pend = (which, t, xT_, src)
            transpose_part(*pend)
            # dummy exp anchored on the last norm's sqrt output: pulls the
            # exp-table load into the ACT window right after the last sqrt,
            # before the first logits are ready (without the anchor the tile
            # scheduler hoists it to t=0, causing a set reload mid phase A)
            nc.scalar.activation(dummy[:], s_last[:], Act.Exp)

        # ---------------- attention -----------------------------------------
        lg_pool = ctx.enter_context(tc.tile_pool(name="lg", bufs=2, space="PSUM"))

        sp_tiles = {}
        acc_tiles = {}
        pend_hsum = []

        def flush_hsum():
            # deferred PE head-sum so PE never blocks the next exp's logits;
            # the sp tile is created here (not in attn_head) so the bufs=1
            # slot is only requested after the previous finale released it
            while pend_hsum:
                it, dg, Et, first, last = pend_hsum.pop(0)
                if it not in sp_tiles:
                    sp_tiles[it] = sp_pool.tile([128, LK], f32, tag="sp",
                                                name=f"sp{it}")
                sp = sp_tiles[it]
                for half in range(2):
                    cols = slice(half * 512, half * 512 + 512)
                    nc.tensor.matmul(
                        sp[:, cols], lhsT=dg[:], rhs=Et[:, cols],
                        start=first, stop=last,
                    )

        def attn_head(it, h):
            o, po = h // 2, (h % 2) * 64
            icols = slice(it * 128, (it + 1) * 128)
            lg = lg_pool.tile([128, LK], f32, tag="lg")
            for half in range(2):
                cols = slice(half * 512, half * 512 + 512)
                nc.tensor.matmul(
                    lg[:, cols],
                    lhsT=qT[po:po + 64, o, icols],
                    rhs=kT[po:po + 64, o, cols],
                    start=True, stop=True,
                )
            flush_hsum()
            Et = et_pool.tile([128, LK], f16, tag="et")
            rs = stats.tile([128, 1], f32, tag="rs")
            escale = einv[it][:] if fold_q else 1.0 / np.sqrt(HD)
            nc.scalar.activation(Et[:], lg[:], Act.Exp,
                                 scale=escale, accum_out=rs[:])
            w = stats.tile([128, 1], f32, tag="w")
            nc.vector.reciprocal(w[:], rs[:])
            if h % 2 == 0 and h not in (2, 4, 6, 10):
                # most even heads: fp16 diag matmul into PSUM scores (on PE);
                # heads 4,6,10 join the DVE accumulation path instead, which
                # unloads PE during the projection-overlapped first block
                # (h==0 keeps the PSUM start flag, h==14 the stop flag)
                dg = dg_pool.tile([128, 128], f16, tag="dg")
                nc.vector.tensor_scalar(
                    out=dg[:], in0=id16[:], scalar1=w[:], scalar2=1.0 / H,
                    op0=Alu.mult, op1=Alu.mult,
                )
                pend_hsum.append((it, dg, Et, h == 0, h == H - 2))
                if h == H - 2:
                    flush_hsum()
            else:
                # odd heads: accumulate w_h*E_h elementwise. The fused DVE
                # scalar_tensor_tensor gets no 2x mode (~1127ns), so for the
                # it1/it3 chains split it: DVE tensor_scalar (2x, ~690ns)
                # then the add on the otherwise-idle GPSIMD engine
                prev = acc_tiles.get(it)
                if prev is None:
                    acc = acc_pool.tile([128, LK], f16, tag="acc")
                    nc.vector.tensor_scalar(
                        out=acc[:], in0=Et[:], scalar1=w[:], scalar2=None,
                        op0=Alu.mult,
                    )
                elif it in (1, 2):
                    wet = wet_pool.tile([128, LK], f16, tag="wet")
                    nc.vector.tensor_scalar(
                        out=wet[:], in0=Et[:], scalar1=w[:], scalar2=None,
                        op0=Alu.mult,
                    )
                    acc = acc_pool.tile([128, LK], f16, tag="acc")
                    nc.gpsimd.tensor_tensor(
                        out=acc[:], in0=wet[:], in1=prev[:], op=Alu.add)
                else:
                    acc = acc_pool.tile([128, LK], f16, tag="acc")
                    nc.vector.scalar_tensor_tensor(
                        out=acc[:], in0=Et[:], scalar=w[:], in1=prev[:],
                        op0=Alu.mult, op1=Alu.add,
                    )
                acc_tiles[it] = acc

        fin_state = {}

        def finale_part1(it):
            icols = slice(it * 128, (it + 1) * 128)
            sp = sp_tiles.pop(it)
            acc = acc_tiles.pop(it)
            scs = scs_pool.tile([128, LK], bf16, tag="scs")
            nc.vector.scalar_tensor_tensor(
                out=scs[:], in0=acc[:], scalar=1.0 / H, in1=sp[:],
                op0=Alu.mult, op1=Alu.add,
            )
            nc.sync.dma_start(sc[icols, :], scs[:])
            # PE transposes beat the high-latency DMA xbar here (measured:
            # the xbar path costs +15us from HWDGE serialization)
            tpP = tpF_pool.tile([128, 8, 128], bf16, tag="ov", name=f"tpP{it}")
            for j in range(8):
                nc.tensor.transpose(
                    tpP[:, j, :], scs[:, j * 128:(j + 1) * 128], id_sb[:])
            scT = scT_pool.tile([128, 8, 128], bf16, tag="scT")
            nc.vector.tensor_copy(scT[:], tpP[:])
            fin_state[it] = {"scT": scT, "ov": None}

        def emit_sv(it, jj):
            # one j-chunk of the attn_output matmul; jj in 0..15
            st = fin_state[it]
            icols = slice(it * 128, (it + 1) * 128)
            half, j = divmod(jj, 8)
            cols = slice(half * 512, half * 512 + 512)
            if j == 0:
                st["ov"] = tpF_pool.tile([128, 512], f32, tag="ov",
                                         name=f"ov{it}_{half}")
            nc.tensor.matmul(
                st["ov"][:], lhsT=st["scT"][:, j, :], rhs=v_sb[:, j, cols],
                start=(j == 0), stop=(j == 7),
            )
            if j == 7:
                osb = osb_pool.tile([128, 512], bf16, tag="osb")
                nc.vector.tensor_copy(osb[:], st["ov"][:])
                nc.sync.dma_start(out[icols, cols], osb[:])
                if half == 1:
                    fin_state.pop(it)

        def finale_last(it):
            # tail version: merge+transpose pipelined per half, attn_output
            # matmuls in 256-col quarters so the last output DMA only waits
            # on the last quarter
            icols = slice(it * 128, (it + 1) * 128)
            sp = sp_tiles.pop(it)
            acc = acc_tiles.pop(it)
            scs = scs_pool.tile([128, LK], bf16, tag="scs")
            tpS = tpF_pool.tile([128, 8, 128], bf16, tag="ov")
            scT = scT_pool.tile([128, 8, 128], bf16, tag="scT")
            # merge+transpose pipelined in 256-col quarters (the serial DVE
            # merges overlap earlier quarters' PE transposes), then the
            # attn_output matmuls in output-column quarters
            for quarter in range(4):
                mcols = slice(quarter * 256, quarter * 256 + 256)
                nc.vector.scalar_tensor_tensor(
                    out=scs[:, mcols], in0=acc[:, mcols], scalar=1.0 / H,
                    in1=sp[:, mcols], op0=Alu.mult, op1=Alu.add,
                )
                for j in range(2 * quarter, 2 * quarter + 2):
                    nc.tensor.transpose(
                        tpS[:, j, :], scs[:, j * 128:(j + 1) * 128], id_sb[:])
                nc.vector.tensor_copy(scT[:, 2 * quarter:2 * quarter + 2, :],
                                      tpS[:, 2 * quarter:2 * quarter + 2, :])
                nc.sync.dma_start(sc[icols, mcols], scs[:, mcols])
            for quarter in range(4):
                cols = slice(quarter * 256, quarter * 256 + 256)
                ov = tpF_pool.tile([128, 256], f32, tag="ov",
                                   name=f"ovl{quarter % 2}")
                for j in range(8):
                    nc.tensor.matmul(
                        ov[:], lhsT=scT[:, j, :], rhs=v_sb[:, j, cols],
                        start=(j == 0), stop=(j == 7),
                    )
                osb = osb_pool.tile([128, 256], bf16, tag="osb")
                nc.vector.tensor_copy(osb[:], ov[:])
                nc.sync.dma_start(out[icols, cols], osb[:])

        # it0 interleaved with remaining projections. Odd heads accumulate in
        # SBUF (no PSUM scores tile), so odd heads of LATER blocks are
        # "floating" work borrowed here to fill the projection-starved gaps
        # in the ACT exp stream.
        with ExitStack() as bctx:
            pools["pp"] = bctx.enter_context(
                tc.tile_pool(name="ppB", bufs=2, space="PSUM"))
            # remaining projection work as a fine-grained chunk queue, dripped
            # 2 chunks per head so PE never lumps ~5us of projections while
            # the exp stream starves
            proj_work = []
            for o in range(2, 8):
                for t in range(8):
                    proj_work.append(("k", o, t))
                for t in range(4):
                    proj_work.append(("q", o, t))

            def drip_proj(n):
                while n > 0 and proj_work:
                    which, o, t = proj_work.pop(0)
                    proj_chunk(o, which, t)
                    if t % 4 == 3:
                        proj_evac(o, which, t // 4)
                    n -= 1

            def need_proj(o):
                # drain until projections for o-tile o are fully emitted
                while any(w[1] <= o for w in proj_work):
                    drip_proj(1)

            def head2(it, h):
                need_proj(h // 2)
                attn_head(it, h)
                drip_proj(2)

            for p in range(8):
                head2(0, 2 * p)
                head2(0, 2 * p + 1)
                head2(1, 2 * p + 1)
                head2(2, 2 * p + 1)
                if 2 * p in (2, 4, 6, 10):
                    # these heads are DVE-accumulated, i.e. PSUM-free
                    # floating work: borrow it1/it2's copies here too
                    head2(1, 2 * p)
                    head2(2, 2 * p)
            drip_proj(len(proj_work))

        load_tiled(v_sb, vv, 0, 4)
        load_tiled(v_sb, vv, 4, 8)

        with ExitStack() as fctx:
            tpF_pool = fctx.enter_context(
                tc.tile_pool(name="tpF", bufs=2, space="PSUM"))
            # later blocks run their even (PSUM) heads, interleaved with the
            # remaining floating odd heads of it3 and the previous block's
            # spread-out attn_output matmuls
            for it in range(1, 4):
                evens = [h for h in range(2, H, 2)
                         if it == 3 or h not in (2, 4, 6, 10)]
                attn_head(it, 0)
                finale_part1(it - 1)
                sv_next = 0
                for i in range(max(len(evens), 4)):
                    if i < len(evens):
                        attn_head(it, evens[i])
                    # it3's floating odd heads all run in the it1 section so
                    # the acc3 chain (DVE) completes long before the finale
                    if it == 1 and i < 4:
                        attn_head(3, 2 * i + 1)
                        attn_head(3, 2 * i + 9)
                    # scT comes from a ~5us-latency DMA transpose; start the
                    # spread attn_output matmuls once it's landed
                    if i >= 1:
                        for _ in range((16 + len(evens) - 2) // max(len(evens) - 1, 1)):
                            if sv_next < 16:
                                emit_sv(it - 1, sv_next)
                                sv_next += 1

                while sv_next < 16:
                    emit_sv(it - 1, sv_next)
                    sv_next += 1
            finale_last(3)

    nc.compile()
    return nc


def _get_program(fold_q=True):
    key = ("nc", bool(fold_q))
    if key not in _CACHE:
        _CACHE[key] = _build_program(fold_q)
    return _CACHE[key]


def kernel(query, key, value, gq, gk, Wq, bq, Wk, bk):
    import ml_dtypes
    from concourse.bass_utils import run_bass_kernel_spmd

    fold_q = bool(np.all(np.asarray(bq) == 0.0))
    nc = _get_program(fold_q)
    bf16 = ml_dtypes.bfloat16

    def pack_w(W, g):
        wt = (np.asarray(W) * np.asarray(g)[None, :]).T.astype(bf16)
        # [e, o] -> [p, o-tile, e-tile, c] with e = e-tile*128+p, o = o-tile*128+c
        return np.ascontiguousarray(
            wt.reshape(8, 128, 8, 128).transpose(1, 2, 0, 3))

    wqt = pack_w(Wq, gq)
    wkt = pack_w(Wk, gk)
    bq2 = np.ascontiguousarray(np.asarray(bq, dtype=np.float32).reshape(8, 128).T)
    bk2 = np.ascontiguousarray(np.asarray(bk, dtype=np.float32).reshape(8, 128).T)
    ident = np.eye(128, dtype=np.float32).astype(bf16)

    q_b = np.asarray(query).astype(bf16)
    k_b = np.asarray(key).astype(bf16)
    v_b = np.asarray(value).astype(bf16)

    in_maps = []
    for c in range(N_CORES):
        b, half = divmod(c, 2)
        i0 = half * QROWS
        in_maps.append({
            "xq": np.ascontiguousarray(q_b[b, i0: i0 + QROWS]),
            "xk": np.ascontiguousarray(k_b[b]),
            "vv": np.ascontiguousarray(v_b[b]),
            "wqt": wqt, "wkt": wkt, "bq": bq2, "bk": bk2, "ident": ident,
        })

    res = run_bass_kernel_spmd(nc, in_maps, list(range(N_CORES)))

    attn_output = np.empty((B, LQ, E), dtype=np.float32)
    attn_scores = np.empty((B, LQ, LK), dtype=np.float32)
    for c in range(N_CORES):
        b, half = divmod(c, 2)
        i0 = half * QROWS
        attn_output[b, i0: i0 + QROWS] = res.results[c]["out"].astype(np.float32)
        attn_scores[b, i0: i0 + QROWS] = res.results[c]["sc"].astype(np.float32)
    return attn_output, attn_scores
